# revision 14
# baseline (speedup 1.0000x reference)
"""TRN2 Bass kernel for nn_AIHWKITSNN: 2-layer leaky-integrate-and-fire SNN.

reference semantics (snntorch Leaky, reset_mechanism='subtract'):
    spikes = (u < clip(x,0,1)),  u = jax.random.uniform(key(42), [T,B,784])
    per step: cur1 = s_t @ W1 + b1; m1 = 0.9*m1 + cur1 - H(m1_prev>1)
              spk1 = H(m1>1); cur2 = spk1 @ W2 + b2; m2 likewise; out spk2

Strategy:
- data-parallel over 8 cores (512 batch rows each), no collectives
- u is input-independent: reproduced on host with the exact jax call the
  reference makes (env default PRNG is rbg and is backend-dependent, so no
  platform pinning), fed to each core as a DRAM input, compared on-device
- layout [hidden, batch]: lhsT = W k-tiles, rhs = spike k-tiles
- weights split W = hi(fp16) + lo(fp16, mostly subnormal -- TensorE
  multiplies subnormals exactly); spikes are exact in bf16 (mixed 16-bit
  matmul operands are supported); products accumulate in fp32 PSUM ->
  near-fp32 matmul precision (~3e-8 abs weight error)
- bias rows folded into the matmuls (u=-1 row -> always-on input; hidden
  unit 500 driven always-on to carry b2)
- LIF recurrence with reset-reuse: rst_t == spk_{t-1}, computed as
  STT (0.9*m + cur), TT (- spk_prev), TS (> 1) matching reference rounding
"""

import numpy as np
from contextlib import ExitStack

import concourse.tile as tile
from concourse import bacc, mybir
from concourse.bass_utils import run_bass_kernel_spmd

F32 = mybir.dt.float32
BF16 = mybir.dt.bfloat16
FP16 = mybir.dt.float16
ALU = mybir.AluOpType

NUM_STEPS = 25
BETA = 0.9
N_CORES = 8
B_FULL, B_LOC = 4096, 512
K_IN = 784            # input pixels
K = K_IN + 1          # + bias row
KT = 7                # ceil(785/128)
K_LAST = K - 6 * 128  # 17 rows in last k-tile
KDUP = K + K_LAST     # last k-tile duplicated so both weight pieces fold
                      # into one K=34 matmul (rows 785:802 copy rows 768:785)
H_REAL, H = 500, 512  # hidden (padded); unit 500 = always-on bias carrier
HT = 4
O, OPAD = 10, 48  # cols 0:16 hi piece, 32:48 lo piece (32-aligned)


# -------------------------------------------------------------------- RNG --
def _gen_u(shape, dtype):
    """Reproduce the reference's spike randomness bit-exactly: same jax call,
    same default device/PRNG as the grading environment's reference run."""
    import jax
    import jax.numpy as jnp
    spike_key = jax.random.key(42)
    u = jax.random.uniform(spike_key, shape, dtype=dtype)
    return np.asarray(u)


# ------------------------------------------------------------------- build --
def _build_nc(reps=1):
    nc = bacc.Bacc()
    u_ext = nc.dram_tensor("u", [NUM_STEPS, KDUP, B_LOC], F32,
                           kind="ExternalInput")
    xT_ext = nc.dram_tensor("xT", [KDUP, B_LOC], F32, kind="ExternalInput")
    w1a_ext = nc.dram_tensor("w1a", [KT, 128, H], FP16, kind="ExternalInput")
    w1b_ext = nc.dram_tensor("w1b", [KT, 128, H], FP16, kind="ExternalInput")
    w1f_ext = nc.dram_tensor("w1f", [2 * K_LAST, H], FP16, kind="ExternalInput")
    w2p_ext = nc.dram_tensor("w2p", [HT, 128, OPAD], FP16, kind="ExternalInput")
    out_ext = nc.dram_tensor("out", [NUM_STEPS, O, B_LOC], F32,
                             kind="ExternalOutput")

    def krows(k):  # valid rows in k-tile k (last: both pieces stacked)
        return 2 * K_LAST if k == KT - 1 else 128

    with tile.TileContext(nc) as tc, ExitStack() as ctx:
        const = ctx.enter_context(tc.tile_pool(name="const", bufs=1))
        state = ctx.enter_context(tc.tile_pool(name="state", bufs=1))
        up = ctx.enter_context(tc.tile_pool(name="up", bufs=3))
        sp = ctx.enter_context(tc.tile_pool(name="sp", bufs=3))
        spkp = ctx.enter_context(tc.tile_pool(name="spkp", bufs=3))
        spk2p = ctx.enter_context(tc.tile_pool(name="spk2p", bufs=2))
        ps1 = ctx.enter_context(tc.tile_pool(name="ps1", bufs=6, space="PSUM"))
        ps2 = ctx.enter_context(tc.tile_pool(name="ps2", bufs=2, space="PSUM"))

        w1a_sb = const.tile([128, KT, H], FP16, tag="w1a")
        w1b_sb = const.tile([128, KT, H], FP16, tag="w1b")
        w1f_sb = const.tile([128, H], FP16, tag="w1f")
        w2p_sb = const.tile([128, HT, OPAD], FP16, tag="w2p")
        xT_sb = const.tile([128, KT, B_LOC], F32, tag="xT")

        m1 = state.tile([128, HT * B_LOC], F32, tag="m1")
        tmp1 = state.tile([128, HT * B_LOC], F32, tag="tmp1")
        m2 = state.tile([16, B_LOC], F32, tag="m2")
        tmp2 = state.tile([16, B_LOC], F32, tag="tmp2")
        for rep in range(reps):
          nc.vector.memset(m1[:], 0.0)
          nc.vector.memset(m2[:], 0.0)
          spk_prev = spkp.tile([128, HT * B_LOC], BF16, tag="spk")
          nc.vector.memset(spk_prev[:], 0.0)
          spk2_prev = spk2p.tile([16, B_LOC], F32, tag="spk2")
          nc.vector.memset(spk2_prev[:], 0.0)
          neg1 = const.tile([128, 1], F32, tag="neg1")
          nc.vector.memset(neg1[:], -1.0)
          zero = const.tile([128, 1], F32, tag="zero")
          nc.vector.memset(zero[:], 0.0)

          def load_spikegen(t, prologue=False):
              u_sb = up.tile([128, KT, B_LOC], F32, tag="u")
              s_sb = sp.tile([128, KT, B_LOC], BF16, tag="s")
              for k in range(KT):
                  r = krows(k)
                  nc.sync.dma_start(u_sb[:r, k, :],
                                    u_ext[t, 128 * k:128 * k + r, :])
                  if prologue:
                      # interleave weight/x loads so k-tile 0 is ready fast
                      nc.sync.dma_start(xT_sb[:r, k, :],
                                        xT_ext[128 * k:128 * k + r, :])
                      if k < KT - 1:
                          nc.sync.dma_start(w1a_sb[:, k, :], w1a_ext[k])
                          nc.sync.dma_start(w1b_sb[:, k, :], w1b_ext[k])
                  nc.vector.tensor_tensor(out=s_sb[:r, k, :],
                                          in0=u_sb[:r, k, :],
                                          in1=xT_sb[:r, k, :], op=ALU.is_lt)
              if prologue:
                  nc.sync.dma_start(w1f_sb[:2 * K_LAST, :], w1f_ext[:, :])
                  for k in range(HT):
                      nc.sync.dma_start(w2p_sb[:, k, :], w2p_ext[k])
              return s_sb

          def layer2_and_out(t, spk):
              p2 = ps2.tile([OPAD, B_LOC], F32, tag="psL2")
              for k in range(HT):
                  bs = slice(B_LOC * k, B_LOC * (k + 1))
                  nc.tensor.matmul(p2[:], w2p_sb[:, k, :], spk[:, bs],
                                   start=(k == 0), stop=(k == HT - 1))
              nonlocal spk2_prev
              lo2 = spk2p.tile([16, B_LOC], F32, tag="lo2")
              nc.scalar.copy(lo2[:], p2[32:48, :])
              nc.vector.scalar_tensor_tensor(tmp2[:], m2[:], BETA,
                                             p2[0:16, :], ALU.mult, ALU.add)
              nc.vector.tensor_tensor(out=tmp2[:], in0=tmp2[:], in1=lo2[:],
                                      op=ALU.add)
              nc.vector.tensor_tensor(out=m2[:], in0=tmp2[:], in1=spk2_prev[:],
                                      op=ALU.subtract)
              spk2 = spk2p.tile([16, B_LOC], F32, tag="spk2")
              nc.vector.tensor_scalar(out=spk2[:], in0=m2[:], scalar1=1.0,
                                      scalar2=None, op0=ALU.is_gt)
              nc.sync.dma_start(out_ext[t], spk2[0:O, :])
              spk2_prev = spk2

          if rep == 0:
              # warm the PE clock gate during the DMA lead-in: ~3.5us of
              # tiny matmuls on zeroed state so real matmuls start at 2.4GHz
              warm = ps2.tile([16, B_LOC], F32, tag="psL2")
              for i in range(20):
                  nc.tensor.matmul(warm[0:16, 0:64], m1[0:128, 0:16],
                                   m1[0:128, 0:64], start=(i == 0),
                                   stop=(i == 19))
          s_cur = load_spikegen(0, prologue=(rep == 0))
          for t in range(NUM_STEPS):
              s_sb = s_cur
              if t + 1 < NUM_STEPS:
                  s_cur = load_spikegen(t + 1)

              # ---- layer 1 matmuls: cur1[h,b] accumulated in PSUM ----
              pls = []
              for h in range(HT):
                  pl = ps1.tile([128, B_LOC], F32, tag="psL1")
                  hs = slice(128 * h, 128 * (h + 1))
                  n = 0
                  for k in range(KT - 1):
                      for wsb in (w1a_sb, w1b_sb):
                          nc.tensor.matmul(pl[:], wsb[:, k, hs],
                                           s_sb[:, k, :], start=(n == 0),
                                           stop=False)
                          n += 1
                  nc.tensor.matmul(pl[:], w1f_sb[:2 * K_LAST, hs],
                                   s_sb[:2 * K_LAST, KT - 1, :],
                                   start=False, stop=True)
                  pls.append(pl)

              # ---- layer 2 of previous step (PE fills the LIF window) ----
              if t > 0:
                  layer2_and_out(t - 1, spk_prev)

              # ---- LIF layer 1 (matches reference rounding order) ----
              for h in range(HT):
                  bs = slice(B_LOC * h, B_LOC * (h + 1))
                  nc.vector.scalar_tensor_tensor(tmp1[:, bs], m1[:, bs], BETA,
                                                 pls[h][:], ALU.mult, ALU.add)
              last = (t == NUM_STEPS - 1)
              spk = spkp.tile([128, HT * B_LOC], BF16, tag="spk")
              if last:
                  # tail: keep the whole chain on DVE (shortest latency)
                  nc.vector.tensor_tensor(out=m1[:], in0=tmp1[:],
                                          in1=spk_prev[:], op=ALU.subtract)
                  nc.vector.tensor_scalar(out=spk[:], in0=m1[:], scalar1=1.0,
                                          scalar2=None, op0=ALU.is_gt)
              else:
                  nc.gpsimd.tensor_tensor(out=m1[:], in0=tmp1[:],
                                          in1=spk_prev[:], op=ALU.subtract)
                  sgn = spkp.tile([128, HT * B_LOC], BF16, tag="sgn")
                  nc.scalar.activation(sgn[:], m1[:],
                                       mybir.ActivationFunctionType.Sign,
                                       bias=neg1[:, :], scale=1.0)
                  nc.scalar.activation(spk[:], sgn[:],
                                       mybir.ActivationFunctionType.Relu,
                                       bias=zero[:, :])
              spk_prev = spk

          layer2_and_out(NUM_STEPS - 1, spk_prev)

    nc.compile()
    return nc


_NC_CACHE = {}


def _get_nc(reps=1):
    if reps not in _NC_CACHE:
        _NC_CACHE[reps] = _build_nc(reps)
    return _NC_CACHE[reps]


# -------------------------------------------------------------- host logic --
def _split2fp16(w):
    """w ~= hi(fp16) + lo(fp16, mostly subnormal; PE multiplies subnormals
    exactly) -- residual ~3e-8 abs for |w|~0.05; measured rel err 0.010."""
    hi = w.astype(np.float16)
    lo = (w.astype(np.float64) - hi.astype(np.float64)).astype(np.float32)
    return hi, lo.astype(np.float16)


def _prepare_inputs(x, W1, b1, W2, b2):
    x = np.asarray(x, np.float32)
    W1 = np.asarray(W1, np.float32)
    b1 = np.asarray(b1, np.float32)
    W2 = np.asarray(W2, np.float32)
    b2 = np.asarray(b2, np.float32)

    u = _gen_u((NUM_STEPS, B_FULL, K_IN), x.dtype)

    # padded weight blocks (shared across cores)
    W1p = np.zeros((KT * 128, H), np.float32)
    W1p[:K_IN, :H_REAL] = W1
    W1p[K_IN, :H_REAL] = b1          # bias row (spike row always 1)
    W1p[K_IN, H_REAL] = 2.0          # drives hidden unit 500 always-on
    w1a, w1b = _split2fp16(W1p)
    w1f = np.concatenate([w1a[6 * 128:K], w1b[6 * 128:K]], axis=0)
    w1f = np.ascontiguousarray(w1f)
    w1a = np.ascontiguousarray(w1a.reshape(KT, 128, H))
    w1b = np.ascontiguousarray(w1b.reshape(KT, 128, H))

    W2f = np.zeros((HT * 128, 16), np.float32)
    W2f[:H_REAL, :O] = W2
    W2f[H_REAL, :O] = b2             # carried by always-on unit 500
    w2hi, w2lo = _split2fp16(W2f)
    w2p = np.zeros((HT * 128, OPAD), np.float16)
    w2p[:, 0:16] = w2hi
    w2p[:, 32:48] = w2lo
    w2p = np.ascontiguousarray(w2p.reshape(HT, 128, OPAD))

    xc = np.clip(x, 0.0, 1.0)
    in_maps = []
    for c in range(N_CORES):
        bs = slice(c * B_LOC, (c + 1) * B_LOC)
        u_core = np.empty((NUM_STEPS, KDUP, B_LOC), np.float32)
        u_core[:, :K_IN, :] = u[:, bs, :].transpose(0, 2, 1)
        u_core[:, K_IN, :] = -1.0    # bias row: always below x -> spike 1
        u_core[:, K:KDUP, :] = u_core[:, 6 * 128:K, :]   # k6 duplicate
        xT_core = np.zeros((KDUP, B_LOC), np.float32)
        xT_core[:K_IN, :] = xc[bs, :].T
        xT_core[K:KDUP, :] = xT_core[6 * 128:K, :]
        # bias row of xT stays 0.0 (-1 < 0 -> spike)
        in_maps.append({
            "u": u_core, "xT": xT_core,
            "w1a": w1a, "w1b": w1b, "w1f": w1f, "w2p": w2p,
        })
    return in_maps


def _run(inputs, trace=False, **kw):
    in_maps = _prepare_inputs(**inputs)
    nc = _get_nc()
    res = None
    for attempt in range(3):
        try:
            res = run_bass_kernel_spmd(nc, in_maps,
                                       core_ids=list(range(N_CORES)),
                                       trace=trace, **kw)
            break
        except Exception:
            if attempt == 2:
                raise
            import time as _time
            _time.sleep(5.0)
    outs = [r["out"] for r in res.results]  # each [T, 10, 512]
    full = np.concatenate([o.transpose(0, 2, 1) for o in outs], axis=1)
    return full.astype(np.float32), res


def kernel(**inputs):
    out, _ = _run(inputs, trace=False)
    return out



# revision 15
# speedup vs baseline: 1.5907x; 1.5907x over previous
"""TRN2 Bass kernel for nn_AIHWKITSNN: 2-layer leaky-integrate-and-fire SNN.

reference semantics (snntorch Leaky, reset_mechanism='subtract'):
    spikes = (u < clip(x,0,1)),  u = jax.random.uniform(key(42), [T,B,784])
    per step: cur1 = s_t @ W1 + b1; m1 = 0.9*m1 + cur1 - H(m1_prev>1)
              spk1 = H(m1>1); cur2 = spk1 @ W2 + b2; m2 likewise; out spk2

Strategy:
- data-parallel over 8 cores (512 batch rows each), no collectives
- u is input-independent: reproduced on host with the exact jax call the
  reference makes (env default PRNG is rbg and is backend-dependent, so no
  platform pinning), fed to each core as a DRAM input, compared on-device
- layout [hidden, batch]: lhsT = W k-tiles, rhs = spike k-tiles
- weights split W = hi(fp16) + lo(fp16, mostly subnormal -- TensorE
  multiplies subnormals exactly); spikes are exact in bf16 (mixed 16-bit
  matmul operands are supported); products accumulate in fp32 PSUM ->
  near-fp32 matmul precision (~3e-8 abs weight error)
- bias rows folded into the matmuls (u=-1 row -> always-on input; hidden
  unit 500 driven always-on to carry b2)
- LIF recurrence with reset-reuse: rst_t == spk_{t-1}, computed as
  STT (0.9*m + cur), TT (- spk_prev), TS (> 1) matching reference rounding
"""

import numpy as np
from contextlib import ExitStack

import concourse.tile as tile
from concourse import bacc, mybir
from concourse.bass_utils import run_bass_kernel_spmd

F32 = mybir.dt.float32
BF16 = mybir.dt.bfloat16
FP16 = mybir.dt.float16
ALU = mybir.AluOpType

NUM_STEPS = 25
BETA = 0.9
N_CORES = 8
B_FULL, B_LOC = 4096, 512
K_IN = 784            # input pixels
K = K_IN + 1          # + bias row
KT = 7                # ceil(785/128)
K_LAST = K - 6 * 128  # 17 rows in last k-tile
KDUP = K + K_LAST     # last k-tile duplicated so both weight pieces fold
                      # into one K=34 matmul (rows 785:802 copy rows 768:785)
H_REAL, H = 500, 512  # hidden (padded); unit 500 = always-on bias carrier
HT = 4
O, OPAD = 10, 48  # cols 0:16 hi piece, 32:48 lo piece (32-aligned)


# -------------------------------------------------------------------- RNG --
def _gen_u(shape, dtype):
    """Reproduce the reference's spike randomness bit-exactly: same jax call,
    same default device/PRNG as the grading environment's reference run."""
    import jax
    import jax.numpy as jnp
    spike_key = jax.random.key(42)
    u = jax.random.uniform(spike_key, shape, dtype=dtype)
    return np.asarray(u)


# ------------------------------------------------------------------- build --
def _build_nc(reps=1):
    nc = bacc.Bacc()
    u_ext = nc.dram_tensor("u", [NUM_STEPS, KDUP, B_LOC], F32,
                           kind="ExternalInput")
    xT_ext = nc.dram_tensor("xT", [KDUP, B_LOC], F32, kind="ExternalInput")
    w1a_ext = nc.dram_tensor("w1a", [KT, 128, H], FP16, kind="ExternalInput")
    w1b_ext = nc.dram_tensor("w1b", [KT, 128, H], FP16, kind="ExternalInput")
    w1f_ext = nc.dram_tensor("w1f", [2 * K_LAST, H], FP16, kind="ExternalInput")
    w2p_ext = nc.dram_tensor("w2p", [HT, 128, OPAD], FP16, kind="ExternalInput")
    out_ext = nc.dram_tensor("out", [NUM_STEPS, O, B_LOC], F32,
                             kind="ExternalOutput")

    def krows(k):  # valid rows in k-tile k (last: both pieces stacked)
        return 2 * K_LAST if k == KT - 1 else 128

    with tile.TileContext(nc) as tc, ExitStack() as ctx:
        const = ctx.enter_context(tc.tile_pool(name="const", bufs=1))
        state = ctx.enter_context(tc.tile_pool(name="state", bufs=1))
        up = ctx.enter_context(tc.tile_pool(name="up", bufs=3))
        sp = ctx.enter_context(tc.tile_pool(name="sp", bufs=3))
        spkp = ctx.enter_context(tc.tile_pool(name="spkp", bufs=3))
        spk2p = ctx.enter_context(tc.tile_pool(name="spk2p", bufs=2))
        ps1 = ctx.enter_context(tc.tile_pool(name="ps1", bufs=6, space="PSUM"))
        ps2 = ctx.enter_context(tc.tile_pool(name="ps2", bufs=2, space="PSUM"))

        w1a_sb = const.tile([128, KT, H], FP16, tag="w1a")
        w1b_sb = const.tile([128, KT, H], FP16, tag="w1b")
        w1f_sb = const.tile([128, H], FP16, tag="w1f")
        w2p_sb = const.tile([128, HT, OPAD], FP16, tag="w2p")
        xT_sb = const.tile([128, KT, B_LOC], F32, tag="xT")

        m1 = state.tile([128, HT * B_LOC], F32, tag="m1")
        tmp1 = state.tile([128, HT * B_LOC], F32, tag="tmp1")
        m2 = state.tile([16, B_LOC], F32, tag="m2")
        tmp2 = state.tile([16, B_LOC], F32, tag="tmp2")
        for rep in range(reps):
          nc.vector.memset(m1[:], 0.0)
          nc.vector.memset(m2[:], 0.0)
          spk_prev = spkp.tile([128, HT * B_LOC], BF16, tag="spk")
          nc.vector.memset(spk_prev[:], 0.0)
          spk2_prev = spk2p.tile([16, B_LOC], F32, tag="spk2")
          nc.vector.memset(spk2_prev[:], 0.0)
          neg1 = const.tile([128, 1], F32, tag="neg1")
          nc.vector.memset(neg1[:], -1.0)
          zero = const.tile([128, 1], F32, tag="zero")
          nc.vector.memset(zero[:], 0.0)

          def load_spikegen(t, prologue=False):
              u_sb = up.tile([128, KT, B_LOC], F32, tag="u")
              s_sb = sp.tile([128, KT, B_LOC], BF16, tag="s")
              for k in range(KT):
                  r = krows(k)
                  nc.sync.dma_start(u_sb[:r, k, :],
                                    u_ext[t, 128 * k:128 * k + r, :])
                  if prologue:
                      # interleave weight/x loads so k-tile 0 is ready fast
                      nc.sync.dma_start(xT_sb[:r, k, :],
                                        xT_ext[128 * k:128 * k + r, :])
                      if k < KT - 1:
                          nc.sync.dma_start(w1a_sb[:, k, :], w1a_ext[k])
                          nc.sync.dma_start(w1b_sb[:, k, :], w1b_ext[k])
                  nc.vector.tensor_tensor(out=s_sb[:r, k, :],
                                          in0=u_sb[:r, k, :],
                                          in1=xT_sb[:r, k, :], op=ALU.is_lt)
              if prologue:
                  nc.sync.dma_start(w1f_sb[:2 * K_LAST, :], w1f_ext[:, :])
                  for k in range(HT):
                      nc.sync.dma_start(w2p_sb[:, k, :], w2p_ext[k])
              return s_sb

          def layer2_and_out(t, spk):
              p2 = ps2.tile([OPAD, B_LOC], F32, tag="psL2")
              for k in range(HT):
                  bs = slice(B_LOC * k, B_LOC * (k + 1))
                  nc.tensor.matmul(p2[:], w2p_sb[:, k, :], spk[:, bs],
                                   start=(k == 0), stop=(k == HT - 1))
              nonlocal spk2_prev
              lo2 = spk2p.tile([16, B_LOC], F32, tag="lo2")
              nc.scalar.copy(lo2[:], p2[32:48, :])
              nc.vector.scalar_tensor_tensor(tmp2[:], m2[:], BETA,
                                             p2[0:16, :], ALU.mult, ALU.add)
              nc.vector.tensor_tensor(out=tmp2[:], in0=tmp2[:], in1=lo2[:],
                                      op=ALU.add)
              nc.vector.tensor_tensor(out=m2[:], in0=tmp2[:], in1=spk2_prev[:],
                                      op=ALU.subtract)
              spk2 = spk2p.tile([16, B_LOC], F32, tag="spk2")
              nc.vector.tensor_scalar(out=spk2[:], in0=m2[:], scalar1=1.0,
                                      scalar2=None, op0=ALU.is_gt)
              nc.sync.dma_start(out_ext[t], spk2[0:O, :])
              spk2_prev = spk2

          if rep == 0:
              # warm the PE clock gate during the DMA lead-in: ~3.5us of
              # tiny matmuls on zeroed state so real matmuls start at 2.4GHz
              warm = ps2.tile([16, B_LOC], F32, tag="psL2")
              for i in range(20):
                  nc.tensor.matmul(warm[0:16, 0:64], m1[0:128, 0:16],
                                   m1[0:128, 0:64], start=(i == 0),
                                   stop=(i == 19))
          s_cur = load_spikegen(0, prologue=(rep == 0))
          for t in range(NUM_STEPS):
              s_sb = s_cur
              if t + 1 < NUM_STEPS:
                  s_cur = load_spikegen(t + 1)

              # ---- layer 1 matmuls: cur1[h,b] accumulated in PSUM ----
              pls = []
              for h in range(HT):
                  pl = ps1.tile([128, B_LOC], F32, tag="psL1")
                  hs = slice(128 * h, 128 * (h + 1))
                  n = 0
                  for k in range(KT - 1):
                      for wsb in (w1a_sb, w1b_sb):
                          nc.tensor.matmul(pl[:], wsb[:, k, hs],
                                           s_sb[:, k, :], start=(n == 0),
                                           stop=False)
                          n += 1
                  nc.tensor.matmul(pl[:], w1f_sb[:2 * K_LAST, hs],
                                   s_sb[:2 * K_LAST, KT - 1, :],
                                   start=False, stop=True)
                  pls.append(pl)

              # ---- layer 2 of previous step (PE fills the LIF window) ----
              if t > 0:
                  layer2_and_out(t - 1, spk_prev)

              # ---- LIF layer 1 (matches reference rounding order) ----
              for h in range(HT):
                  bs = slice(B_LOC * h, B_LOC * (h + 1))
                  nc.vector.scalar_tensor_tensor(tmp1[:, bs], m1[:, bs], BETA,
                                                 pls[h][:], ALU.mult, ALU.add)
              last = (t == NUM_STEPS - 1)
              spk = spkp.tile([128, HT * B_LOC], BF16, tag="spk")
              if last:
                  # tail: keep the whole chain on DVE (shortest latency)
                  nc.vector.tensor_tensor(out=m1[:], in0=tmp1[:],
                                          in1=spk_prev[:], op=ALU.subtract)
                  nc.vector.tensor_scalar(out=spk[:], in0=m1[:], scalar1=1.0,
                                          scalar2=None, op0=ALU.is_gt)
              else:
                  nc.vector.tensor_tensor(out=m1[:], in0=tmp1[:],
                                          in1=spk_prev[:], op=ALU.subtract)
                  sgn = spkp.tile([128, HT * B_LOC], BF16, tag="sgn")
                  nc.scalar.activation(sgn[:], m1[:],
                                       mybir.ActivationFunctionType.Sign,
                                       bias=neg1[:, :], scale=1.0)
                  nc.scalar.activation(spk[:], sgn[:],
                                       mybir.ActivationFunctionType.Relu,
                                       bias=zero[:, :])
              spk_prev = spk

          layer2_and_out(NUM_STEPS - 1, spk_prev)

    nc.compile()
    return nc


_NC_CACHE = {}


def _get_nc(reps=1):
    if reps not in _NC_CACHE:
        _NC_CACHE[reps] = _build_nc(reps)
    return _NC_CACHE[reps]


# -------------------------------------------------------------- host logic --
def _split2fp16(w):
    """w ~= hi(fp16) + lo(fp16, mostly subnormal; PE multiplies subnormals
    exactly) -- residual ~3e-8 abs for |w|~0.05; measured rel err 0.010."""
    hi = w.astype(np.float16)
    lo = (w.astype(np.float64) - hi.astype(np.float64)).astype(np.float32)
    return hi, lo.astype(np.float16)


def _prepare_inputs(x, W1, b1, W2, b2):
    x = np.asarray(x, np.float32)
    W1 = np.asarray(W1, np.float32)
    b1 = np.asarray(b1, np.float32)
    W2 = np.asarray(W2, np.float32)
    b2 = np.asarray(b2, np.float32)

    u = _gen_u((NUM_STEPS, B_FULL, K_IN), x.dtype)

    # padded weight blocks (shared across cores)
    W1p = np.zeros((KT * 128, H), np.float32)
    W1p[:K_IN, :H_REAL] = W1
    W1p[K_IN, :H_REAL] = b1          # bias row (spike row always 1)
    W1p[K_IN, H_REAL] = 2.0          # drives hidden unit 500 always-on
    w1a, w1b = _split2fp16(W1p)
    w1f = np.concatenate([w1a[6 * 128:K], w1b[6 * 128:K]], axis=0)
    w1f = np.ascontiguousarray(w1f)
    w1a = np.ascontiguousarray(w1a.reshape(KT, 128, H))
    w1b = np.ascontiguousarray(w1b.reshape(KT, 128, H))

    W2f = np.zeros((HT * 128, 16), np.float32)
    W2f[:H_REAL, :O] = W2
    W2f[H_REAL, :O] = b2             # carried by always-on unit 500
    w2hi, w2lo = _split2fp16(W2f)
    w2p = np.zeros((HT * 128, OPAD), np.float16)
    w2p[:, 0:16] = w2hi
    w2p[:, 32:48] = w2lo
    w2p = np.ascontiguousarray(w2p.reshape(HT, 128, OPAD))

    xc = np.clip(x, 0.0, 1.0)
    in_maps = []
    for c in range(N_CORES):
        bs = slice(c * B_LOC, (c + 1) * B_LOC)
        u_core = np.empty((NUM_STEPS, KDUP, B_LOC), np.float32)
        u_core[:, :K_IN, :] = u[:, bs, :].transpose(0, 2, 1)
        u_core[:, K_IN, :] = -1.0    # bias row: always below x -> spike 1
        u_core[:, K:KDUP, :] = u_core[:, 6 * 128:K, :]   # k6 duplicate
        xT_core = np.zeros((KDUP, B_LOC), np.float32)
        xT_core[:K_IN, :] = xc[bs, :].T
        xT_core[K:KDUP, :] = xT_core[6 * 128:K, :]
        # bias row of xT stays 0.0 (-1 < 0 -> spike)
        in_maps.append({
            "u": u_core, "xT": xT_core,
            "w1a": w1a, "w1b": w1b, "w1f": w1f, "w2p": w2p,
        })
    return in_maps


def _run(inputs, trace=False, **kw):
    in_maps = _prepare_inputs(**inputs)
    nc = _get_nc()
    res = None
    for attempt in range(3):
        try:
            res = run_bass_kernel_spmd(nc, in_maps,
                                       core_ids=list(range(N_CORES)),
                                       trace=trace, **kw)
            break
        except Exception:
            if attempt == 2:
                raise
            import time as _time
            _time.sleep(5.0)
    outs = [r["out"] for r in res.results]  # each [T, 10, 512]
    full = np.concatenate([o.transpose(0, 2, 1) for o in outs], axis=1)
    return full.astype(np.float32), res


def kernel(**inputs):
    out, _ = _run(inputs, trace=False)
    return out

